# revision 1
# baseline (speedup 1.0000x reference)
"""Trainium2 Bass kernel for nn_CrossAttention_28183575396415.

The reference block-mask gives every query exactly one key (kv = q_idx // 3),
so the softmax weight is identically 1 and the q/k projections, RMSNorm and
RoPE are dead code.  The module reduces to

    out[b, t] = x_kv[b, t // 3] @ Wv.T @ Wproj.T
              = x_kv[b, t // 3] @ WfT          with WfT = Wv.T @ Wproj.T

Strategy (8 NeuronCores, SPMD):
  - Host folds the two projection matrices into WfT (computed in float64,
    stored float32) — constant folding of adjacent linear layers.
  - The 4*2048 = 8192 kv rows are row-sharded 8 ways (1024 rows/core).
    Each core's shard is pre-transposed on host so every device DMA is a
    natural contiguous load and the PE needs no on-device transposes; the
    shard and the weight are concatenated into one [1024(k), 2048] input so
    each k-tile arrives in a single 1 MiB DMA:
        xw[:, :1024]  = x_shard.T   (k on partitions = contraction dim)
        xw[:, 1024:]  = WfT
  - Device: z = xT.T @ WfT with K accumulated in PSUM (8 k-tiles), then each
    z row tile is written to HBM three times (the t//3 replication), giving
    this core's contiguous [3072, 1024] slice of the flattened output.
  - Host unshard = concatenate the 8 slices.
"""

import json
import os

import numpy as np

import concourse.bass as bass
import concourse.mybir as mybir
from concourse.tile import TileContext
from concourse.vector_clock import ScopedClock
from concourse.bass_utils import run_bass_kernel_spmd

P = 128          # partitions
C = 1024         # model dim
K_T = C // P     # k tiles
M_T = C // P     # row tiles per core shard
N = 512          # matmul free dim (one PSUM bank of fp32)
L = 3            # replication factor (Tq // Tkv)
ROWS_PER_CORE = 1024
N_CORES = 8

# compute dtype: "f32r" (full-rate fp32 PE mode), "bf16", or "f32" (4x slower)
COMPUTE_DT = os.environ.get("KERNEL_COMPUTE_DT", "f32r")
# "device3": device writes the replicated [3072, 1024] slice (default)
# "host1":   device writes [1024, 1024]; host repeats rows (debug/compare only)
OUT_MODE = os.environ.get("KERNEL_OUT_MODE", "device3")


class SlimTailTileContext(TileContext):
    """Tile's kernel tail is drain -> barrier -> ~280 serialized per-semaphore
    clear instructions -> barrier (~8 us measured).  The clears only matter if
    the loaded NEFF executes more than once; every kernel() call here builds a
    fresh jit executable (fresh NEFF load, semaphores re-initialized), so skip
    them and the second barrier.  The drain still waits for every DMA queue,
    so outputs are complete before the program ends."""

    def _drain_and_barrier(self, tick_clock, wait_clock):
        # The SP drain (with its hoisted wait chain) already gates on every
        # engine's clock and every DMA queue, so outputs are complete when SP
        # retires; with no sem-clears to order, the closing all-engine
        # barrier adds nothing but latency.
        drain_inst = self.nc.sync.drain()
        wait_clock.add_sem_waits(
            drain_inst.ins, ScopedClock({None: tick_clock.global_clock})
        )
        popped = self.nc._tile_sem_poison_stack.pop()
        assert popped is self._sem_poison


def _split_multiwaits(nc: bass.Bass) -> None:
    """This container's walrus allows only ONE sync-wait on several
    instruction formats (Drain/CTRL, Matmult's LDWEIGHTS half, ...).  Tile
    can emit more.  Post-pass the serialized BIR: for any instruction with
    >1 on_wait, hoist all but the last wait onto single-wait EventSemaphore
    carriers inserted immediately before it on the same engine (waits then
    execute in queue order — semantics unchanged).  The patched JSON is
    pinned on the instance so every downstream serialization sees it."""
    raw = bass.Bass.to_json_bytes(nc)
    j = json.loads(raw)
    n_hoisted = 0
    for f in j["functions"]:
        for bb in f["blocks"]:
            new_insts = []
            for ins in bb["instructions"]:
                si = ins.get("sync_info")
                waits = si.get("on_wait", []) if si else []
                if len(waits) > 1:
                    for i, w in enumerate(waits[:-1]):
                        carrier = {
                            "engine": ins["engine"],
                            "ins": [],
                            "outs": [],
                            "name": f"{ins['name']}_hw{i}",
                            "opcode": "EventSemaphore",
                            "sync_info": {"on_update": [], "on_wait": [w]},
                        }
                        if "debug" in ins:
                            carrier["debug"] = ins["debug"]
                        new_insts.append(carrier)
                        n_hoisted += 1
                    si["on_wait"] = waits[-1:]
                new_insts.append(ins)
            bb["instructions"] = new_insts
    patched = json.dumps(j).encode()
    nc.to_json_bytes = lambda: patched


def _build(compute_dt: str, out_mode: str) -> bass.Bass:
    nc = bass.Bass("TRN2")
    in_mydt = {
        "bf16": mybir.dt.bfloat16,
        "f32r": mybir.dt.float32r,
        "f32": mybir.dt.float32,
    }[compute_dt]

    W2 = ROWS_PER_CORE + C  # concatenated [x | w] free dim
    xw = nc.dram_tensor("xw", [C, W2], in_mydt, kind="ExternalInput")
    n_rep = L if out_mode == "device3" else 1
    out = nc.dram_tensor(
        "out", [n_rep * ROWS_PER_CORE, C], mybir.dt.float32, kind="ExternalOutput"
    )

    xw_t = xw.rearrange("(t p) m -> t p m", p=P)  # [8, 128, 2048]
    # out row (n_rep*g + r) <- z row g
    out_rep = out.rearrange("(g r) c -> g r c", r=n_rep)  # [1024, n_rep, 1024]

    with SlimTailTileContext(nc) as tc:
        with (
            tc.tile_pool(name="xw", bufs=1) as xw_pool,
            tc.tile_pool(name="psum", bufs=8, space="PSUM") as psum_pool,
            tc.tile_pool(name="zout", bufs=6) as z_pool,
        ):
            # Load two k-tiles per DMA (2 MiB each, side by side in the free
            # dim) and alternate the trigger engine so the input stream isn't
            # paced by a single engine's ~1us-per-trigger issue cost.
            # First k-tile alone (1 MiB) so the PE can start as early as
            # possible; the rest in 2 MiB pair-DMAs to amortize trigger cost.
            in_eng = [nc.sync, nc.scalar]
            groups = [[0], [1, 2], [3, 4], [5, 6], [7]]
            xwk = [None] * K_T
            for j, grp in enumerate(groups):
                n = len(grp)
                t = xw_pool.tile([P, n * W2], in_mydt, name=f"xwp{j}", tag=f"xwp{j}")
                src = xw[grp[0] * P : (grp[0] + n) * P, :].rearrange(
                    "(g p) m -> p g m", p=P
                )
                dst = t[:].rearrange("p (g m) -> p g m", g=n)
                in_eng[j % 2].dma_start(dst, src)
                for i, k in enumerate(grp):
                    xwk[k] = (t, i * W2)

            # Two passes over the output-column halves.  Each pass keeps one
            # PSUM bank per row-tile (8 banks), accumulates over k in lockstep
            # with the input DMA stream, and its evictions/stores start right
            # after the last input byte — so the output DMA stream begins as
            # early as the data dependency allows and the two passes keep the
            # DMA engines saturated end-to-end.
            evict_eng = [
                lambda dst, src: nc.vector.tensor_copy(dst, src),
                lambda dst, src: nc.vector.tensor_copy(dst, src),
            ]
            out_eng = [nc.sync, nc.scalar]
            for cc in range(2):
                ps = [
                    psum_pool.tile([P, N], mybir.dt.float32, name=f"ps{cc}_{m}", tag="ps")
                    for m in range(M_T)
                ]
                for k in range(K_T):
                    tile_k, off = xwk[k]
                    rhs = tile_k[
                        :, off + ROWS_PER_CORE + cc * N : off + ROWS_PER_CORE + (cc + 1) * N
                    ]
                    for m in range(M_T):
                        nc.tensor.matmul(
                            ps[m][:],
                            tile_k[:, off + m * P : off + (m + 1) * P],
                            rhs,
                            start=(k == 0),
                            stop=(k == K_T - 1),
                        )
                for m in range(M_T):
                    zh = z_pool.tile([P, N], mybir.dt.float32, name=f"z{cc}_{m}", tag="z")
                    evict_eng[m % 2](zh[:], ps[m][:])
                    for r in range(n_rep):
                        out_eng[(m * n_rep + r) % 2].dma_start(
                            out_rep[m * P : (m + 1) * P, r, cc * N : (cc + 1) * N],
                            zh[:],
                        )

    _split_multiwaits(nc)
    return nc


_NC_CACHE: dict = {}


def _get_nc(compute_dt: str, out_mode: str) -> bass.Bass:
    key = (compute_dt, out_mode)
    if key not in _NC_CACHE:
        _NC_CACHE[key] = _build(compute_dt, out_mode)
    return _NC_CACHE[key]


def kernel(x_q, x_kv, Wq, Wk, Wv, Wproj, _compute_dt=None, _out_mode=None):
    compute_dt = _compute_dt or COMPUTE_DT
    out_mode = _out_mode or OUT_MODE
    B, Tkv, C_ = x_kv.shape
    assert (B, Tkv, C_) == (4, 2048, C)

    # Fold the two projections: z = x @ Wv.T @ Wproj.T = x @ WfT
    WfT = (Wv.astype(np.float64).T @ Wproj.astype(np.float64).T).astype(np.float32)

    x_flat = x_kv.reshape(B * Tkv, C)
    in_maps = []
    for c in range(N_CORES):
        shard = x_flat[c * ROWS_PER_CORE : (c + 1) * ROWS_PER_CORE]
        xw = np.concatenate([shard.T, WfT], axis=1)  # [C(k), 2048]
        if compute_dt == "bf16":
            import ml_dtypes

            xw = xw.astype(ml_dtypes.bfloat16)
        else:
            xw = np.ascontiguousarray(xw)
        in_maps.append({"xw": xw})

    nc = _get_nc(compute_dt, out_mode)
    res = run_bass_kernel_spmd(nc, in_maps, core_ids=list(range(N_CORES)))

    Tq = L * Tkv
    blocks = []
    for c in range(N_CORES):
        blk = res.results[c]["out"]
        if out_mode != "device3":
            blk = np.repeat(blk, L, axis=0)
        blocks.append(blk)
    out_flat = np.concatenate(blocks, axis=0)  # [B*Tq, C]
    return out_flat.reshape(B, Tq, C)



# revision 3
# speedup vs baseline: 1.7668x; 1.7668x over previous
"""Trainium2 Bass kernel for nn_CrossAttention_28183575396415.

The reference block-mask gives every query exactly one key (kv = q_idx // 3),
so the softmax weight is identically 1 and the q/k projections, RMSNorm and
RoPE are dead code.  The module reduces to

    out[b, t] = x_kv[b, t // 3] @ Wv.T @ Wproj.T
              = x_kv[b, t // 3] @ WfT          with WfT = Wv.T @ Wproj.T

Strategy (8 NeuronCores, SPMD):
  - Host folds the two projection matrices into WfT (computed in float64)
    - constant folding of adjacent linear layers.
  - The 4*2048 = 8192 kv rows are row-sharded 8 ways (1024 rows/core).
    Each core's shard is pre-transposed on host so every device DMA is a
    natural contiguous load; the shard and the weight are concatenated into
    one [1024(k), 2048] bf16 input so each k-tile arrives in a single DMA:
        xw[:, :1024]  = x_shard.T   (k on partitions = contraction dim)
        xw[:, 1024:]  = WfT
  - Device: z = xT.T @ WfT, K accumulated in PSUM over 8 k-tiles.
    Column half 0 runs k-major (overlapping the input stream), column half 1
    runs m-major so finished row tiles retire one at a time and the output
    DMA stream stays busy until the end instead of bursting at the tail.
  - Each z tile is written to HBM with a single DMA whose source AP repeats
    the tile 3x (stride-0 middle dim) - the t//3 replication - giving this
    core's contiguous [3072, 1024] slice of the flattened output in bf16.
  - Host unshard = concatenate the 8 slices and upcast to float32.
"""

import json
import os

import numpy as np

import concourse.bass as bass
import concourse.mybir as mybir
from bass_rust import AP
from concourse.tile import TileContext
from concourse.vector_clock import ScopedClock
from concourse.bass_utils import run_bass_kernel_spmd

P = 128          # partitions
C = 1024         # model dim
K_T = C // P     # k tiles
M_T = C // P     # row tiles per core shard
N = 512          # matmul free dim (one PSUM bank of fp32)
L = 3            # replication factor (Tq // Tkv)
ROWS_PER_CORE = 1024
N_CORES = 8

# compute dtype: "bf16" (half the input DMA), "f32r"/"f32" for debugging
COMPUTE_DT = os.environ.get("KERNEL_COMPUTE_DT", "bf16")
# output dtype on device: "bf16" (host upcasts) or "f32"
OUT_DT = os.environ.get("KERNEL_OUT_DT", "bf16")
# "bcast": one DMA per z tile with stride-0 replication; "multi": 3 DMAs
REP_MODE = os.environ.get("KERNEL_REP_MODE", "bcast")


class SlimTailTileContext(TileContext):
    """Tile's kernel tail is drain -> barrier -> ~280 serialized per-semaphore
    clear instructions -> barrier (~8 us measured).  The clears only matter if
    the loaded NEFF executes more than once; every kernel() call here builds a
    fresh jit executable (fresh NEFF load, semaphores re-initialized), so skip
    them and the second barrier.  The drain still waits for every DMA queue,
    so outputs are complete before the program ends."""

    def _drain_and_barrier(self, tick_clock, wait_clock):
        drain_inst = self.nc.sync.drain()
        wait_clock.add_sem_waits(
            drain_inst.ins, ScopedClock({None: tick_clock.global_clock})
        )
        popped = self.nc._tile_sem_poison_stack.pop()
        assert popped is self._sem_poison


def _split_multiwaits(nc: bass.Bass) -> None:
    """This container's walrus allows only ONE sync-wait on several
    instruction formats (Drain/CTRL, Matmult's LDWEIGHTS half, ...).  Tile
    can emit more.  Post-pass the serialized BIR: for any instruction with
    >1 on_wait, hoist all but the last wait onto single-wait EventSemaphore
    carriers inserted immediately before it on the same engine (waits then
    execute in queue order - semantics unchanged)."""
    raw = bass.Bass.to_json_bytes(nc)
    j = json.loads(raw)
    for f in j["functions"]:
        for bb in f["blocks"]:
            new_insts = []
            for ins in bb["instructions"]:
                si = ins.get("sync_info")
                waits = si.get("on_wait", []) if si else []
                if len(waits) > 1:
                    for i, w in enumerate(waits[:-1]):
                        carrier = {
                            "engine": ins["engine"],
                            "ins": [],
                            "outs": [],
                            "name": f"{ins['name']}_hw{i}",
                            "opcode": "EventSemaphore",
                            "sync_info": {"on_update": [], "on_wait": [w]},
                        }
                        if "debug" in ins:
                            carrier["debug"] = ins["debug"]
                        new_insts.append(carrier)
                    si["on_wait"] = waits[-1:]
                new_insts.append(ins)
            bb["instructions"] = new_insts
    patched = json.dumps(j).encode()
    nc.to_json_bytes = lambda: patched


def _rep3_src(zh_ap):
    """Source AP reading a [P, N] SBUF tile as [P, L, N] via a stride-0
    middle dim - the DMA replicates each row L times."""
    lay = zh_ap.ap
    assert len(lay) == 2, lay
    return AP(tensor=zh_ap.tensor, offset=zh_ap.offset, ap=[lay[0], [0, L], lay[1]])


def _build(compute_dt: str, out_dt: str, rep_mode: str) -> bass.Bass:
    nc = bass.Bass("TRN2")
    in_mydt = {
        "bf16": mybir.dt.bfloat16,
        "f32r": mybir.dt.float32r,
        "f32": mybir.dt.float32,
    }[compute_dt]
    out_mydt = {"bf16": mybir.dt.bfloat16, "f32": mybir.dt.float32}[out_dt]

    W2 = ROWS_PER_CORE + C  # concatenated [x | w] free dim
    xw = nc.dram_tensor("xw", [C, W2], in_mydt, kind="ExternalInput")
    out = nc.dram_tensor(
        "out", [L * ROWS_PER_CORE, C], out_mydt, kind="ExternalOutput"
    )
    # out row (L*g + r) <- z row g
    out_rep = out.rearrange("(g r) c -> g r c", r=L)  # [1024, L, 1024]

    with SlimTailTileContext(nc) as tc:
        with (
            tc.tile_pool(name="xw", bufs=1) as xw_pool,
            tc.tile_pool(name="psum", bufs=8, space="PSUM") as psum_pool,
            tc.tile_pool(name="zout", bufs=16) as z_pool,
        ):
            # Input: first k-tile alone so the PE starts as early as possible,
            # the rest in pair-DMAs to amortize trigger cost.  Alternate the
            # trigger engine so one engine's issue cost doesn't pace the
            # stream.
            in_eng = [nc.sync, nc.scalar]
            groups = [[0], [1, 2], [3, 4], [5, 6], [7]]
            xwk = [None] * K_T
            for j, grp in enumerate(groups):
                n = len(grp)
                t = xw_pool.tile([P, n * W2], in_mydt, name=f"xwp{j}", tag=f"xwp{j}")
                src = xw[grp[0] * P : (grp[0] + n) * P, :].rearrange(
                    "(g p) m -> p g m", p=P
                )
                dst = t[:].rearrange("p (g m) -> p g m", g=n)
                in_eng[j % 2].dma_start(dst, src)
                for i, k in enumerate(grp):
                    xwk[k] = (t, i * W2)

            out_eng = [nc.sync, nc.scalar]
            n_trig = [0]

            def store(zh, m, cc):
                dst = out_rep[m * P : (m + 1) * P, :, cc * N : (cc + 1) * N]
                if rep_mode == "bcast":
                    out_eng[n_trig[0] % 2].dma_start(dst, _rep3_src(zh[:]))
                    n_trig[0] += 1
                else:
                    for r in range(L):
                        out_eng[n_trig[0] % 2].dma_start(
                            out_rep[m * P : (m + 1) * P, r, cc * N : (cc + 1) * N],
                            zh[:],
                        )
                        n_trig[0] += 1

            # Pass 0 (columns 0:512): k-major, in lockstep with the input
            # stream.  Evictions alternate vector/scalar to clear the
            # end-of-pass burst quickly.
            ps0 = [
                psum_pool.tile([P, N], mybir.dt.float32, name=f"ps0_{m}", tag="ps")
                for m in range(M_T)
            ]
            for k in range(K_T):
                tile_k, off = xwk[k]
                rhs = tile_k[:, off + ROWS_PER_CORE : off + ROWS_PER_CORE + N]
                for m in range(M_T):
                    nc.tensor.matmul(
                        ps0[m][:],
                        tile_k[:, off + m * P : off + (m + 1) * P],
                        rhs,
                        start=(k == 0),
                        stop=(k == K_T - 1),
                    )
            evict0 = [nc.vector.tensor_copy, nc.scalar.copy]
            for m in range(M_T):
                zh = z_pool.tile([P, N], out_mydt, name=f"z0_{m}", tag="z")
                evict0[m % 2](zh[:], ps0[m][:])
                store(zh, m, 0)

            # Pass 1 (columns 512:1024): m-major so each row tile finishes
            # 1.7us after the previous one and its output DMA streams
            # immediately - no end-of-kernel output burst.
            for m in range(M_T):
                ps = psum_pool.tile([P, N], mybir.dt.float32, name=f"ps1_{m}", tag="ps")
                for k in range(K_T):
                    tile_k, off = xwk[k]
                    nc.tensor.matmul(
                        ps[:],
                        tile_k[:, off + m * P : off + (m + 1) * P],
                        tile_k[:, off + ROWS_PER_CORE + N : off + ROWS_PER_CORE + 2 * N],
                        start=(k == 0),
                        stop=(k == K_T - 1),
                    )
                zh = z_pool.tile([P, N], out_mydt, name=f"z1_{m}", tag="z")
                nc.vector.tensor_copy(zh[:], ps[:])
                store(zh, m, 1)

    _split_multiwaits(nc)
    return nc


_NC_CACHE: dict = {}


def _get_nc(compute_dt: str, out_dt: str, rep_mode: str) -> bass.Bass:
    key = (compute_dt, out_dt, rep_mode)
    if key not in _NC_CACHE:
        _NC_CACHE[key] = _build(compute_dt, out_dt, rep_mode)
    return _NC_CACHE[key]


def kernel(x_q, x_kv, Wq, Wk, Wv, Wproj, _compute_dt=None, _out_dt=None):
    compute_dt = _compute_dt or COMPUTE_DT
    out_dt = _out_dt or OUT_DT
    B, Tkv, C_ = x_kv.shape
    assert (B, Tkv, C_) == (4, 2048, C)

    # Fold the two projections: z = x @ Wv.T @ Wproj.T = x @ WfT
    WfT = (Wv.astype(np.float64).T @ Wproj.astype(np.float64).T).astype(np.float32)

    x_flat = x_kv.reshape(B * Tkv, C)
    in_maps = []
    for c in range(N_CORES):
        shard = x_flat[c * ROWS_PER_CORE : (c + 1) * ROWS_PER_CORE]
        xw = np.concatenate([shard.T, WfT], axis=1)  # [C(k), 2048]
        if compute_dt == "bf16":
            import ml_dtypes

            xw = xw.astype(ml_dtypes.bfloat16)
        else:
            xw = np.ascontiguousarray(xw)
        in_maps.append({"xw": xw})

    nc = _get_nc(compute_dt, out_dt, REP_MODE)
    res = run_bass_kernel_spmd(nc, in_maps, core_ids=list(range(N_CORES)))

    Tq = L * Tkv
    blocks = [res.results[c]["out"] for c in range(N_CORES)]
    out_flat = np.concatenate(blocks, axis=0)  # [B*Tq, C]
    return out_flat.reshape(B, Tq, C).astype(np.float32)
